# revision 39
# baseline (speedup 1.0000x reference)
"""AugmentedLSTMCell on 8 TRN2 NeuronCores — data-parallel over batch.

Layout: feature-on-partition (transposed). Per core: B_loc=2048 batch rows.
  proj.T[j, b] = sum_e W[j, e] * in[b, e]
  lhsT tiles  = W.T blocks [128e, 128j]  (host pre-packed)
  rhs         = in.T        [128e, 2048b] (host pre-transposed)
  psum [128j, 2048b] accumulates the Wi-proj and Ws-proj contraction
  (the "fused = proj_in + proj_st" add comes free via PSUM accumulation).
  ScalarE applies per-feature bias + sigmoid/tanh straight out of PSUM.
Host transposes outputs back to [B, H].

Perf structure (PE-bound: every 512-wide matmul instr costs ~216ns
regardless of dtype; fp8 DoubleRow contracts 2 k-tiles per instr = 2x):
  - i/f/o/hw gates run fully fp8 DoubleRow on BOTH sides. The m gate
    (feeds mem directly through tanh, so it owns most of the mem-output
    error budget) runs h-side fully fp8 + x-side fully bf16 — same PE
    cost and numerics as a symmetric 4+4 split, but needs no bf16 h at
    all, cutting 2MB off the startup DMA stream. hwp (the highway
    projection, enters out linearly) stays full bf16. Allocation chosen
    by an exact host-side numerics simulator: sim rel_err out=1.55e-2 /
    mem=1.88e-2 (limit 2e-2); sim matches hardware to ~4 digits.
  - fp8 product scale S = sW*sA is folded out via the activation's
    scale operand; the m gate's bf16 x-weights are pre-scaled by S.
  - DMA discipline: every transfer keeps >=1KB contiguous runs per
    partition (<512B runs halve DMA rate). fp8 activations live in one
    resident tile per k-PAIR (readers of a multi-write tile wait on all
    its writers, so k-pairs get their own tiles and the first matmul
    depends on 512KB, not 2MB). Engine/ring roles: gpsimd streams all
    inputs (fp8 activations first, then weights + c), scalar carries
    the bf16 x tiles then only runs activations (a DMA issue stuck on
    ring credits would delay its activations), sync carries outputs.
  - group-0 gates are computed fp8-first (o,hw,i,f) so only ~1.2MB must
    land before the PE starts; x bf16 (m gate + hwp) streams behind.
  - outputs written as bf16 (halves output HBM traffic; host upcasts)
  - last tile group reordered: hwp computed LAST in per-bc chunks with
    fused blend+DMA so the post-matmul tail is short; its weights are
    prefetched one group early.
"""
import sys
import types

sys.path.insert(0, "/opt/trn_rl_repo")
sys.path.insert(0, "/root/.axon_site")

# Shim antenv.axon_hooks (missing on this image) so trace=True can profile.
if "antenv.axon_hooks" not in sys.modules:
    _hooks = types.ModuleType("antenv.axon_hooks")
    _state = {"hook": None}
    _hooks.set_axon_ntff_profile_hook = lambda h: _state.__setitem__("hook", h)
    _hooks.get_axon_ntff_profile_hook = lambda: _state["hook"]
    sys.modules["antenv.axon_hooks"] = _hooks
    try:
        from trn_agent_boot.trn_boot import _ntff_profile_via_ctypes

        _hooks.set_axon_ntff_profile_hook(
            _ntff_profile_via_ctypes("/opt/axon/libaxon_pjrt.so")
        )
    except Exception:
        pass

import numpy as np
import ml_dtypes

import concourse.bass as bass
import concourse.bacc as bacc
import concourse.mybir as mybir
from concourse import tile
from concourse.bass_utils import run_bass_kernel_spmd

BF16 = ml_dtypes.bfloat16
F8E4 = ml_dtypes.float8_e4m3fn

N_CORES = 8
B, E, H = 16384, 1024, 1024
BL = B // N_CORES          # 2048 batch rows per core
KT = E // 128              # 8 contraction k-tiles
NJI = 6 * H // 128         # 48 feature tiles of proj_in
NJS = 5 * H // 128         # 40 feature tiles of proj_st (the gates)
NT = H // 128              # 8 H-slices
BC = 512                   # matmul moving free dim (one PSUM bank)
NBC = BL // BC             # batch chunks per matmul group

AF = mybir.ActivationFunctionType
DR = mybir.MatmulPerfMode.DoubleRow

# M_MIX of the 8 m-gate tiles run x-side k-tiles 0-1 in fp8 too. At
# M_MIX=4 (9/16 of m fp8, sim mem rel_err 1.9385e-2, confirmed by HW)
# this buys 16 PE slots (~3.5us). A first attempt measured 469us, but
# every matmul — including byte-identical gate streams — ran ~18%
# slower, i.e. a chip downclock, not a code effect; retried.
# M_MIX=5: sim mem rel_err 1.9529e-2 (sim-HW agreement ~0.02% on 7 runs).
M_MIX = 5


def is_m(jt):
    return 2 * NT <= jt < 3 * NT


def build_nc():
    nc = bacc.Bacc(None, target_bir_lowering=False)
    f32, bf16 = mybir.dt.float32, mybir.dt.bfloat16
    f8 = mybir.dt.float8e4

    xT = nc.declare_dram_parameter("xT", [E, BL], bf16, isOutput=False)
    cT = nc.declare_dram_parameter("cT", [H, BL], bf16, isOutput=False)
    whwp = nc.declare_dram_parameter("whwp", [NT, 128, E], bf16, isOutput=False)
    bias = nc.declare_dram_parameter("bias", [128, NJI], f32, isOutput=False)
    outT = nc.declare_dram_parameter("outT", [H, BL], bf16, isOutput=True)
    memT = nc.declare_dram_parameter("memT", [H, BL], bf16, isOutput=True)
    # fp8 weight copies (x side unused for m tiles); m x-side bf16 full-k
    wq8x = nc.declare_dram_parameter("wq8x", [NJS, 128, KT, 128], f8, isOutput=False)
    wq8h = nc.declare_dram_parameter("wq8h", [NJS, 128, KT, 128], f8, isOutput=False)
    wbxm = nc.declare_dram_parameter("wbxm", [NT, 128, KT * 128], bf16, isOutput=False)
    xq8 = nc.declare_dram_parameter("xq8", [128, KT, BL], f8, isOutput=False)
    hq8 = nc.declare_dram_parameter("hq8", [128, KT, BL], f8, isOutput=False)
    scl = nc.declare_dram_parameter("scl", [128, 1], f32, isOutput=False)

    with tile.TileContext(nc) as tc:
        with (
            tc.tile_pool(name="resident", bufs=1) as resident,
            tc.tile_pool(name="wpool", bufs=4) as wpool,
            tc.tile_pool(name="cpool", bufs=2) as cpool,
            tc.tile_pool(name="psum", bufs=2, space="PSUM") as psum_pool,
            tc.tile_pool(name="gates", bufs=9) as gate_pool,
            tc.tile_pool(name="tmp", bufs=4) as tmp_pool,
            tc.tile_pool(name="outp", bufs=4) as out_pool,
        ):
            bias_sb = resident.tile([128, NJI], f32, tag="bias")
            scl_sb = resident.tile([128, 1], f32, tag="scl")

            # one resident tile per fp8 k-pair (dep granularity)
            xq8_kp = [
                resident.tile([128, 2, BL], f8, tag=f"xq8_{j}", name=f"xq8_{j}")
                for j in range(KT // 2)
            ]
            hq8_kp = [
                resident.tile([128, 2, BL], f8, tag=f"hq8_{j}", name=f"hq8_{j}")
                for j in range(KT // 2)
            ]
            xt_k = [
                resident.tile([128, BL], bf16, tag=f"xt{k}", name=f"xt{k}")
                for k in range(KT)
            ]

            def rhs_x(k, bc):
                return xt_k[k][:, bc * BC : (bc + 1) * BC]

            def load_gate_w(jt, eng=None):
                eng = eng or nc.gpsimd
                g8xm = None
                kfx = 8
                if is_m(jt):
                    t = jt - 2 * NT
                    kfx = 2 if t < M_MIX else 0
                    if kfx:
                        g8xm = wpool.tile([128, kfx, 128], f8, tag="w8xm")
                        eng.dma_start(g8xm[:], wq8x[jt][:, :kfx, :])
                    wx = wpool.tile([128, (KT - kfx) * 128], bf16,
                                    tag="wbxm", bufs=3)
                    eng.dma_start(wx[:], wbxm[t][:, kfx * 128 :])
                else:
                    wx = wpool.tile([128, KT, 128], f8, tag="w8x")
                    eng.dma_start(wx[:], wq8x[jt])
                g8h = wpool.tile([128, KT, 128], f8, tag="w8h")
                eng.dma_start(g8h[:], wq8h[jt])
                return (wx, g8xm, g8h, kfx)

            # ---- startup DMA choreography ---------------------------------
            # fp8 k-pairs load whole on the gpsimd ring ([128, 2, BL] = 4KB
            # contiguous per partition, 128 descriptors — splitting them
            # fragments descriptors and slows the stream). First matmuls
            # (o0 x-side j=0) depend only on k-pair 0 + o0's 128KB weights.
            # Startup spreads across all three rings (each credit-stalls at
            # ~1.7us/issue, so no single ring can feed the PE): gpsimd
            # carries x k-pairs + o/hw/i/f weights; scalar leads with h
            # k-pairs 0-1 (needed ~21us) + x bf16 k0-3 (needed ~36us) and
            # is done issuing long before its first activation (~25us);
            # sync carries the small stuff then outputs.
            # Consecutive k-pairs come from ALTERNATING rings so o0 consumes
            # them in arrival order with no per-ring serialization: a single
            # ring delivers a 512KB k-pair only every ~4us (credit stalls),
            # twice the PE's warmup consumption period.
            nc.gpsimd.dma_start(xq8_kp[0][:], xq8[:, 0:2, :])
            nc.scalar.dma_start(xq8_kp[1][:], xq8[:, 2:4, :])
            w_o0 = load_gate_w(3 * NT)           # o gate, tile 0 (fp8-only)
            nc.scalar.dma_start(xq8_kp[2][:], xq8[:, 4:6, :])
            nc.gpsimd.dma_start(xq8_kp[3][:], xq8[:, 6:8, :])
            nc.scalar.dma_start(hq8_kp[0][:], hq8[:, 0:2, :])
            w_hw0 = load_gate_w(4 * NT)          # hw gate, tile 0
            nc.scalar.dma_start(hq8_kp[2][:], hq8[:, 4:6, :])
            nc.gpsimd.dma_start(hq8_kp[1][:], hq8[:, 2:4, :])
            nc.gpsimd.dma_start(hq8_kp[3][:], hq8[:, 6:8, :])
            for k in range(4):
                nc.scalar.dma_start(xt_k[k][:], xT[k * 128 : (k + 1) * 128, :])
            nc.sync.dma_start(bias_sb[:], bias[:])
            nc.sync.dma_start(scl_sb[:], scl[:])
            w_i0 = load_gate_w(0)                # i gate, tile 0
            w_f0 = load_gate_w(NT)               # f gate, tile 0
            # m0 + hwp0 weights go on the (otherwise idle) sync ring: as
            # gpsimd issues #17-20 they would land at ~39-44us, right at
            # their use deadline, stalling the PE at the m0->hwp0 boundary
            # (the gpsimd ring credit-stalls to ~1.7us per issue).
            w_m0 = load_gate_w(2 * NT, eng=nc.sync)
            w_hwp0 = wpool.tile([128, E], bf16, tag="w")
            nc.sync.dma_start(w_hwp0[:], whwp[0])

            def gate_tile(jt, func, w=None, chunk_act=1):
                """Gate proj tile jt (0..NJS-1) -> activated gate (bf16)."""
                if w is None:
                    w = load_gate_w(jt)
                wx, g8xm, g8h, kfx = w
                ps = psum_pool.tile([128, BL], f32, tag="ps")
                if is_m(jt):
                    for j in range(kfx // 2):
                        for bc in range(NBC):
                            lo = bc * BC
                            nc.tensor.matmul(
                                ps[:, lo : lo + BC],
                                g8xm[:, 2 * j : 2 * j + 2, :],
                                xq8_kp[j][:, :, lo : lo + BC],
                                start=(j == 0), stop=False, perf_mode=DR,
                            )
                    for k in range(kfx, KT):
                        lhsT = wx[:, (k - kfx) * 128 : (k - kfx + 1) * 128]
                        for bc in range(NBC):
                            lo = bc * BC
                            nc.tensor.matmul(
                                ps[:, lo : lo + BC], lhsT, rhs_x(k, bc),
                                start=(kfx == 0 and k == 0), stop=False,
                            )
                else:
                    for j in range(KT // 2):
                        for bc in range(NBC):
                            lo = bc * BC
                            nc.tensor.matmul(
                                ps[:, lo : lo + BC],
                                wx[:, 2 * j : 2 * j + 2, :],
                                xq8_kp[j][:, :, lo : lo + BC],
                                start=(j == 0), stop=False, perf_mode=DR,
                            )
                for j in range(KT // 2):
                    for bc in range(NBC):
                        lo = bc * BC
                        nc.tensor.matmul(
                            ps[:, lo : lo + BC],
                            g8h[:, 2 * j : 2 * j + 2, :],
                            hq8_kp[j][:, :, lo : lo + BC],
                            start=False, stop=(j == KT // 2 - 1),
                            perf_mode=DR,
                        )
                g = gate_pool.tile([128, BL], bf16, tag="g")
                cw = BL // chunk_act
                for a in range(chunk_act):
                    sl = slice(a * cw, (a + 1) * cw)
                    nc.scalar.activation(
                        g[:, sl], ps[:, sl], func,
                        bias=bias_sb[:, jt : jt + 1], scale=scl_sb[:, 0:1],
                    )
                return g

            def gate_pair(jt_a, jt_b, func_a, func_b, w_a, w_b):
                """Two fp8-only gate tiles with interleaved matmuls: each
                arriving fp8 k-pair feeds 8 instructions instead of 4, so
                the warmup PE consumption rate matches the HBM-limited
                k-pair arrival rate (uses both PSUM buffers)."""
                wxa, g8ha = w_a
                wxb, g8hb = w_b
                psa = psum_pool.tile([128, BL], f32, tag="ps")
                psb = psum_pool.tile([128, BL], f32, tag="ps")
                for j in range(KT // 2):
                    for ps_, wx_ in ((psa, wxa), (psb, wxb)):
                        for bc in range(NBC):
                            lo = bc * BC
                            nc.tensor.matmul(
                                ps_[:, lo : lo + BC],
                                wx_[:, 2 * j : 2 * j + 2, :],
                                xq8_kp[j][:, :, lo : lo + BC],
                                start=(j == 0), stop=False, perf_mode=DR,
                            )
                for j in range(KT // 2):
                    for ps_, g8h_ in ((psa, g8ha), (psb, g8hb)):
                        for bc in range(NBC):
                            lo = bc * BC
                            nc.tensor.matmul(
                                ps_[:, lo : lo + BC],
                                g8h_[:, 2 * j : 2 * j + 2, :],
                                hq8_kp[j][:, :, lo : lo + BC],
                                start=False, stop=(j == KT // 2 - 1),
                                perf_mode=DR,
                            )
                ga = gate_pool.tile([128, BL], bf16, tag="g")
                nc.scalar.activation(ga[:], psa[:], func_a,
                                     bias=bias_sb[:, jt_a : jt_a + 1],
                                     scale=scl_sb[:, 0:1])
                gb = gate_pool.tile([128, BL], bf16, tag="g")
                nc.scalar.activation(gb[:], psb[:], func_b,
                                     bias=bias_sb[:, jt_b : jt_b + 1],
                                     scale=scl_sb[:, 0:1])
                return ga, gb

            def hwp_tile(t, w_i=None, chunk_act=1):
                """hw_proj tile t (x-only, full bf16, unscaled)."""
                jt = 5 * NT + t
                if w_i is None:
                    w_i = wpool.tile([128, E], bf16, tag="w")
                    nc.gpsimd.dma_start(w_i[:], whwp[t])
                ps = psum_pool.tile([128, BL], f32, tag="ps")
                for k in range(KT):
                    lhsT = w_i[:, k * 128 : (k + 1) * 128]
                    for bc in range(NBC):
                        lo = bc * BC
                        nc.tensor.matmul(
                            ps[:, lo : lo + BC], lhsT, rhs_x(k, bc),
                            start=(k == 0), stop=(k == KT - 1),
                        )
                g = gate_pool.tile([128, BL], bf16, tag="g")
                cw = BL // chunk_act
                for a in range(chunk_act):
                    sl = slice(a * cw, (a + 1) * cw)
                    nc.scalar.activation(
                        g[:, sl], ps[:, sl], AF.Identity,
                        bias=bias_sb[:, jt : jt + 1]
                    )
                return g

            mult, addop, subop = (
                mybir.AluOpType.mult,
                mybir.AluOpType.add,
                mybir.AluOpType.subtract,
            )

            w_hwp_l = None
            for t in range(NT):
                last = t == NT - 1
                if t == 0:
                    # group 0, fp8-first: o/hw/i/f need only the quantized
                    # activations, giving the x bf16 stream (m gate, hwp)
                    # the longest runway.
                    o_g = gate_tile(3 * NT, AF.Sigmoid, w=w_o0)
                    # xt k4-7 (1st needed by m0 ~40us) issue here, between
                    # activations: they stay out of the fp8 stream's way
                    # during the critical 8-28us warmup window.
                    for k in range(4, 6):
                        nc.scalar.dma_start(xt_k[k][:],
                                            xT[k * 128 : (k + 1) * 128, :])
                    hw_g = gate_tile(4 * NT, AF.Sigmoid, w=w_hw0)
                    for k in range(6, KT):
                        nc.scalar.dma_start(xt_k[k][:],
                                            xT[k * 128 : (k + 1) * 128, :])
                    i_g = gate_tile(0, AF.Sigmoid, w=w_i0)
                    f_g = gate_tile(NT, AF.Sigmoid, w=w_f0)
                    m_g = gate_tile(2 * NT, AF.Tanh, w=w_m0)
                else:
                    if not last:
                        hwp = hwp_tile(t)
                    i_g = gate_tile(t, AF.Sigmoid)
                    m_g = gate_tile(2 * NT + t, AF.Tanh)
                    f_g = gate_tile(NT + t, AF.Sigmoid)

                ct = cpool.tile([128, BL], bf16, tag="c")
                nc.gpsimd.dma_start(ct[:], cT[t * 128 : (t + 1) * 128, :])

                t1 = tmp_pool.tile([128, BL], bf16, tag="tmp")
                nc.vector.tensor_tensor(t1[:], i_g[:], m_g[:], mult)
                t2 = tmp_pool.tile([128, BL], bf16, tag="tmp")
                nc.vector.tensor_tensor(t2[:], f_g[:], ct[:], mult)
                mem = out_pool.tile([128, BL], bf16, tag="mem")
                nc.vector.tensor_tensor(mem[:], t1[:], t2[:], addop)
                nc.sync.dma_start(memT[t * 128 : (t + 1) * 128, :], mem[:])

                if t != 0:
                    if last:
                        # hw7 BEFORE o7: its chunked activations drain while
                        # o7's matmuls run, so the hwp blend chunks' psum
                        # allocation never waits on them.
                        hw_g = gate_tile(4 * NT + t, AF.Sigmoid, chunk_act=4)
                    o_g = gate_tile(3 * NT + t, AF.Sigmoid)

                tmem = tmp_pool.tile([128, BL], bf16, tag="tmp")
                nc.scalar.activation(tmem[:], mem[:], AF.Tanh)
                outp = tmp_pool.tile([128, BL], bf16, tag="tmp")
                nc.vector.tensor_tensor(outp[:], o_g[:], tmem[:], mult)

                if t != 0 and not last:
                    hw_g = gate_tile(4 * NT + t, AF.Sigmoid)
                if t == 0:
                    hwp = hwp_tile(0, w_i=w_hwp0)
                if t == NT - 2:
                    # prefetch the last group's hwp weights
                    w_hwp_l = wpool.tile([128, E], bf16, tag="w")
                    nc.gpsimd.dma_start(w_hwp_l[:], whwp[NT - 1])

                if not last:
                    # out = hwp + hw*(outp - hwp), chunked so the blend
                    # pipelines with the output DMA.
                    u = tmp_pool.tile([128, BL], bf16, tag="tmp")
                    nc.vector.tensor_tensor(u[:], outp[:], hwp[:], subop)
                    for e in range(2):
                        sl = slice(e * (BL // 2), (e + 1) * (BL // 2))
                        v = tmp_pool.tile([128, BL // 2], bf16, tag="v")
                        nc.vector.tensor_tensor(v[:], hw_g[:, sl], u[:, sl], mult)
                        outf = out_pool.tile([128, BL // 2], bf16, tag="out")
                        nc.vector.tensor_tensor(outf[:], v[:], hwp[:, sl], addop)
                        nc.sync.dma_start(outT[t * 128 : (t + 1) * 128, sl], outf[:])
                else:
                    # Last group: hwp computed LAST in per-bc chunks with
                    # fused blend+DMA; final 512 cols split into two 256-col
                    # chunks so the post-matmul tail is minimal.
                    jt = 5 * NT + t
                    chunks = [(0, BC), (BC, BC), (2 * BC, BC),
                              (3 * BC, BC // 2), (3 * BC + BC // 2, BC // 2)]
                    for ci, (lo, cw) in enumerate(chunks):
                        sl = slice(lo, lo + cw)
                        ps = psum_pool.tile([128, cw], f32, tag="ps")
                        for k in range(KT):
                            nc.tensor.matmul(
                                ps[:],
                                w_hwp_l[:, k * 128 : (k + 1) * 128],
                                rhs_x(k, lo // BC)[:, lo % BC : lo % BC + cw],
                                start=(k == 0),
                                stop=(k == KT - 1),
                            )
                        # alternate blend lanes vector/gpsimd so the final
                        # chunks' element-wise chains run in parallel. The
                        # hwp bias-add needs a PSUM port (gpsimd has none):
                        # vector serves its own lane, scalar (free by now)
                        # serves the gpsimd lane, so the lanes never
                        # serialize on one engine.
                        ve = nc.vector if ci % 2 == 0 else nc.gpsimd
                        hwp_c = tmp_pool.tile([128, cw], bf16, tag="v")
                        if ci % 2 == 0:
                            nc.vector.tensor_scalar_add(
                                hwp_c[:], ps[:], bias_sb[:, jt : jt + 1]
                            )
                        else:
                            nc.scalar.activation(
                                hwp_c[:], ps[:], AF.Identity,
                                bias=bias_sb[:, jt : jt + 1],
                            )
                        u = tmp_pool.tile([128, cw], bf16, tag="v")
                        ve.tensor_tensor(u[:], outp[:, sl], hwp_c[:], subop)
                        v = tmp_pool.tile([128, cw], bf16, tag="v")
                        ve.tensor_tensor(v[:], hw_g[:, sl], u[:], mult)
                        outf = out_pool.tile([128, cw], bf16, tag="out")
                        ve.tensor_tensor(outf[:], v[:], hwp_c[:], addop)
                        nc.sync.dma_start(outT[t * 128 : (t + 1) * 128, sl], outf[:])

    nc.compile()
    return nc


_NC_CACHE = None


def _get_nc():
    global _NC_CACHE
    if _NC_CACHE is None:
        _NC_CACHE = build_nc()
    return _NC_CACHE


def _pack_weights(W, njt):
    # W [njt*128 j, K e] -> [njt, 128 p, K] with [jt, p, k*128+m] = W[jt*128+m, k*128+p]
    W = np.asarray(W, np.float32)
    K = W.shape[1]
    kt = K // 128
    return np.ascontiguousarray(
        W.reshape(njt, 128, kt, 128).transpose(0, 3, 2, 1).reshape(njt, 128, K)
    )


def _q8(a, sc):
    return np.clip(np.asarray(a, np.float32) * sc, -240.0, 240.0).astype(F8E4)


def prepare_in_maps(x, h, c, Wi, bi, Ws, bs):
    x = np.asarray(x, np.float32)
    h = np.asarray(h, np.float32)
    Wi = np.asarray(Wi, np.float32)
    Ws = np.asarray(Ws, np.float32)
    Wg = Wi[: 5 * H]

    bias_comb = np.concatenate(
        [np.asarray(bi[: 5 * H], np.float32) + np.asarray(bs, np.float32),
         np.asarray(bi[5 * H :], np.float32)]
    )
    bias_pack = np.ascontiguousarray(bias_comb.reshape(NJI, 128).T).astype(np.float32)
    whwp_p = _pack_weights(Wi[5 * H :], NT).astype(BF16)

    sW = np.float32(224.0 / max(np.abs(Wg).max(), np.abs(Ws).max()))
    sA = np.float32(224.0 / max(np.abs(x).max(), np.abs(h).max()))
    S = np.float32(sW * sA)
    # [jt, m, j, p] -> [jt, p, j, m]
    wq8x_p = _q8(Wg.reshape(NJS, 128, KT, 128).transpose(0, 3, 2, 1), sW)
    wq8h_p = _q8(Ws.reshape(NJS, 128, KT, 128).transpose(0, 3, 2, 1), sW)
    wbxm_p = np.ascontiguousarray(
        _pack_weights(Wg[2 * H : 3 * H] * S, NT)).astype(BF16)
    common = {
        "bias": bias_pack,
        "whwp": whwp_p,
        "wq8x": np.ascontiguousarray(wq8x_p),
        "wq8h": np.ascontiguousarray(wq8h_p),
        "wbxm": wbxm_p,
        "scl": np.full((128, 1), 1.0 / S, np.float32),
    }

    in_maps = []
    for i in range(N_CORES):
        s = slice(i * BL, (i + 1) * BL)
        m = {
            "xT": np.ascontiguousarray(x[s].T).astype(BF16),
            "cT": np.ascontiguousarray(np.asarray(c[s], np.float32).T).astype(BF16),
            # [p, j, b] with value in[b, j*128+p]
            "xq8": np.ascontiguousarray(
                _q8(x[s].T.reshape(KT, 128, BL).transpose(1, 0, 2), sA)),
            "hq8": np.ascontiguousarray(
                _q8(h[s].T.reshape(KT, 128, BL).transpose(1, 0, 2), sA)),
        }
        m.update(common)
        in_maps.append(m)
    return in_maps


def run(in_maps, trace=False):
    nc = _get_nc()
    res = run_bass_kernel_spmd(nc, in_maps, core_ids=list(range(N_CORES)), trace=trace)
    out = np.empty((B, H), np.float32)
    mem = np.empty((B, H), np.float32)
    for i in range(N_CORES):
        s = slice(i * BL, (i + 1) * BL)
        out[s] = res.results[i]["outT"].astype(np.float32).T
        mem[s] = res.results[i]["memT"].astype(np.float32).T
    return (out, mem), res


def kernel(x, h, c, Wi, bi, Ws, bs):
    in_maps = prepare_in_maps(x, h, c, Wi, bi, Ws, bs)
    (out, mem), _ = run(in_maps, trace=False)
    return out, mem


# revision 40
# speedup vs baseline: 1.0043x; 1.0043x over previous
"""AugmentedLSTMCell on 8 TRN2 NeuronCores — data-parallel over batch.

Layout: feature-on-partition (transposed). Per core: B_loc=2048 batch rows.
  proj.T[j, b] = sum_e W[j, e] * in[b, e]
  lhsT tiles  = W.T blocks [128e, 128j]  (host pre-packed)
  rhs         = in.T        [128e, 2048b] (host pre-transposed)
  psum [128j, 2048b] accumulates the Wi-proj and Ws-proj contraction
  (the "fused = proj_in + proj_st" add comes free via PSUM accumulation).
  ScalarE applies per-feature bias + sigmoid/tanh straight out of PSUM.
Host transposes outputs back to [B, H].

Perf structure (PE-bound: every 512-wide matmul instr costs ~216ns
regardless of dtype; fp8 DoubleRow contracts 2 k-tiles per instr = 2x):
  - i/f/o/hw gates run fully fp8 DoubleRow on BOTH sides. The m gate
    (feeds mem directly through tanh, so it owns most of the mem-output
    error budget) runs h-side fully fp8 + x-side fully bf16 — same PE
    cost and numerics as a symmetric 4+4 split, but needs no bf16 h at
    all, cutting 2MB off the startup DMA stream. hwp (the highway
    projection, enters out linearly) stays full bf16. Allocation chosen
    by an exact host-side numerics simulator: sim rel_err out=1.55e-2 /
    mem=1.88e-2 (limit 2e-2); sim matches hardware to ~4 digits.
  - fp8 product scale S = sW*sA is folded out via the activation's
    scale operand; the m gate's bf16 x-weights are pre-scaled by S.
  - DMA discipline: every transfer keeps >=1KB contiguous runs per
    partition (<512B runs halve DMA rate). fp8 activations live in one
    resident tile per k-PAIR (readers of a multi-write tile wait on all
    its writers, so k-pairs get their own tiles and the first matmul
    depends on 512KB, not 2MB). Engine/ring roles: gpsimd streams all
    inputs (fp8 activations first, then weights + c), scalar carries
    the bf16 x tiles then only runs activations (a DMA issue stuck on
    ring credits would delay its activations), sync carries outputs.
  - group-0 gates are computed fp8-first (o,hw,i,f) so only ~1.2MB must
    land before the PE starts; x bf16 (m gate + hwp) streams behind.
  - outputs written as bf16 (halves output HBM traffic; host upcasts)
  - last tile group reordered: hwp computed LAST in per-bc chunks with
    fused blend+DMA so the post-matmul tail is short; its weights are
    prefetched one group early.
"""
import sys
import types

sys.path.insert(0, "/opt/trn_rl_repo")
sys.path.insert(0, "/root/.axon_site")

# Shim antenv.axon_hooks (missing on this image) so trace=True can profile.
if "antenv.axon_hooks" not in sys.modules:
    _hooks = types.ModuleType("antenv.axon_hooks")
    _state = {"hook": None}
    _hooks.set_axon_ntff_profile_hook = lambda h: _state.__setitem__("hook", h)
    _hooks.get_axon_ntff_profile_hook = lambda: _state["hook"]
    sys.modules["antenv.axon_hooks"] = _hooks
    try:
        from trn_agent_boot.trn_boot import _ntff_profile_via_ctypes

        _hooks.set_axon_ntff_profile_hook(
            _ntff_profile_via_ctypes("/opt/axon/libaxon_pjrt.so")
        )
    except Exception:
        pass

import numpy as np
import ml_dtypes

import concourse.bass as bass
import concourse.bacc as bacc
import concourse.mybir as mybir
from concourse import tile
from concourse.bass_utils import run_bass_kernel_spmd

BF16 = ml_dtypes.bfloat16
F8E4 = ml_dtypes.float8_e4m3fn

N_CORES = 8
B, E, H = 16384, 1024, 1024
BL = B // N_CORES          # 2048 batch rows per core
KT = E // 128              # 8 contraction k-tiles
NJI = 6 * H // 128         # 48 feature tiles of proj_in
NJS = 5 * H // 128         # 40 feature tiles of proj_st (the gates)
NT = H // 128              # 8 H-slices
BC = 512                   # matmul moving free dim (one PSUM bank)
NBC = BL // BC             # batch chunks per matmul group

AF = mybir.ActivationFunctionType
DR = mybir.MatmulPerfMode.DoubleRow

# M_MIX of the 8 m-gate tiles run x-side k-tiles 0-1 in fp8 too. At
# M_MIX=4 (9/16 of m fp8, sim mem rel_err 1.9385e-2, confirmed by HW)
# this buys 16 PE slots (~3.5us). A first attempt measured 469us, but
# every matmul — including byte-identical gate streams — ran ~18%
# slower, i.e. a chip downclock, not a code effect; retried.
# M_MIX=5: sim mem rel_err 1.9529e-2 (sim-HW agreement ~0.02% on 7 runs).
M_MIX = 5


def is_m(jt):
    return 2 * NT <= jt < 3 * NT


def build_nc():
    nc = bacc.Bacc(None, target_bir_lowering=False)
    f32, bf16 = mybir.dt.float32, mybir.dt.bfloat16
    f8 = mybir.dt.float8e4

    xT = nc.declare_dram_parameter("xT", [E, BL], bf16, isOutput=False)
    cT = nc.declare_dram_parameter("cT", [H, BL], bf16, isOutput=False)
    whwp = nc.declare_dram_parameter("whwp", [NT, 128, E], bf16, isOutput=False)
    bias = nc.declare_dram_parameter("bias", [128, NJI], f32, isOutput=False)
    outT = nc.declare_dram_parameter("outT", [H, BL], bf16, isOutput=True)
    memT = nc.declare_dram_parameter("memT", [H, BL], bf16, isOutput=True)
    # fp8 weight copies (x side unused for m tiles); m x-side bf16 full-k
    wq8x = nc.declare_dram_parameter("wq8x", [NJS, 128, KT, 128], f8, isOutput=False)
    wq8h = nc.declare_dram_parameter("wq8h", [NJS, 128, KT, 128], f8, isOutput=False)
    wbxm = nc.declare_dram_parameter("wbxm", [NT, 128, KT * 128], bf16, isOutput=False)
    xq8 = nc.declare_dram_parameter("xq8", [128, KT, BL], f8, isOutput=False)
    hq8 = nc.declare_dram_parameter("hq8", [128, KT, BL], f8, isOutput=False)
    scl = nc.declare_dram_parameter("scl", [128, 1], f32, isOutput=False)

    with tile.TileContext(nc) as tc:
        with (
            tc.tile_pool(name="resident", bufs=1) as resident,
            tc.tile_pool(name="wpool", bufs=4) as wpool,
            tc.tile_pool(name="cpool", bufs=2) as cpool,
            tc.tile_pool(name="psum", bufs=2, space="PSUM") as psum_pool,
            tc.tile_pool(name="gates", bufs=9) as gate_pool,
            tc.tile_pool(name="tmp", bufs=4) as tmp_pool,
            tc.tile_pool(name="outp", bufs=4) as out_pool,
        ):
            bias_sb = resident.tile([128, NJI], f32, tag="bias")
            scl_sb = resident.tile([128, 1], f32, tag="scl")

            # one resident tile per fp8 k-pair (dep granularity)
            xq8_kp = [
                resident.tile([128, 2, BL], f8, tag=f"xq8_{j}", name=f"xq8_{j}")
                for j in range(KT // 2)
            ]
            hq8_kp = [
                resident.tile([128, 2, BL], f8, tag=f"hq8_{j}", name=f"hq8_{j}")
                for j in range(KT // 2)
            ]
            xt_k = [
                resident.tile([128, BL], bf16, tag=f"xt{k}", name=f"xt{k}")
                for k in range(KT)
            ]

            def rhs_x(k, bc):
                return xt_k[k][:, bc * BC : (bc + 1) * BC]

            def load_gate_w(jt, eng=None):
                eng = eng or nc.gpsimd
                g8xm = None
                kfx = 8
                if is_m(jt):
                    t = jt - 2 * NT
                    kfx = 2 if t < M_MIX else 0
                    if kfx:
                        g8xm = wpool.tile([128, kfx, 128], f8, tag="w8xm")
                        eng.dma_start(g8xm[:], wq8x[jt][:, :kfx, :])
                    wx = wpool.tile([128, (KT - kfx) * 128], bf16,
                                    tag="wbxm", bufs=3)
                    eng.dma_start(wx[:], wbxm[t][:, kfx * 128 :])
                else:
                    wx = wpool.tile([128, KT, 128], f8, tag="w8x")
                    eng.dma_start(wx[:], wq8x[jt])
                g8h = wpool.tile([128, KT, 128], f8, tag="w8h")
                eng.dma_start(g8h[:], wq8h[jt])
                return (wx, g8xm, g8h, kfx)

            # ---- startup DMA choreography ---------------------------------
            # fp8 k-pairs load whole on the gpsimd ring ([128, 2, BL] = 4KB
            # contiguous per partition, 128 descriptors — splitting them
            # fragments descriptors and slows the stream). First matmuls
            # (o0 x-side j=0) depend only on k-pair 0 + o0's 128KB weights.
            # Startup spreads across all three rings (each credit-stalls at
            # ~1.7us/issue, so no single ring can feed the PE): gpsimd
            # carries x k-pairs + o/hw/i/f weights; scalar leads with h
            # k-pairs 0-1 (needed ~21us) + x bf16 k0-3 (needed ~36us) and
            # is done issuing long before its first activation (~25us);
            # sync carries the small stuff then outputs.
            # Consecutive k-pairs come from ALTERNATING rings so o0 consumes
            # them in arrival order with no per-ring serialization: a single
            # ring delivers a 512KB k-pair only every ~4us (credit stalls),
            # twice the PE's warmup consumption period.
            nc.gpsimd.dma_start(xq8_kp[0][:], xq8[:, 0:2, :])
            nc.scalar.dma_start(xq8_kp[1][:], xq8[:, 2:4, :])
            w_o0 = load_gate_w(3 * NT)           # o gate, tile 0 (fp8-only)
            nc.scalar.dma_start(xq8_kp[2][:], xq8[:, 4:6, :])
            nc.gpsimd.dma_start(xq8_kp[3][:], xq8[:, 6:8, :])
            nc.scalar.dma_start(hq8_kp[0][:], hq8[:, 0:2, :])
            w_hw0 = load_gate_w(4 * NT)          # hw gate, tile 0
            nc.scalar.dma_start(hq8_kp[2][:], hq8[:, 4:6, :])
            nc.gpsimd.dma_start(hq8_kp[1][:], hq8[:, 2:4, :])
            nc.gpsimd.dma_start(hq8_kp[3][:], hq8[:, 6:8, :])
            for k in range(4):
                nc.scalar.dma_start(xt_k[k][:], xT[k * 128 : (k + 1) * 128, :])
            nc.sync.dma_start(bias_sb[:], bias[:])
            nc.sync.dma_start(scl_sb[:], scl[:])
            w_i0 = load_gate_w(0)                # i gate, tile 0
            w_f0 = load_gate_w(NT)               # f gate, tile 0
            # m0 + hwp0 weights go on the (otherwise idle) sync ring: as
            # gpsimd issues #17-20 they would land at ~39-44us, right at
            # their use deadline, stalling the PE at the m0->hwp0 boundary
            # (the gpsimd ring credit-stalls to ~1.7us per issue).
            w_m0 = load_gate_w(2 * NT, eng=nc.sync)
            w_hwp0 = wpool.tile([128, E], bf16, tag="w")
            nc.sync.dma_start(w_hwp0[:], whwp[0])

            def gate_tile(jt, func, w=None, chunk_act=1):
                """Gate proj tile jt (0..NJS-1) -> activated gate (bf16)."""
                if w is None:
                    w = load_gate_w(jt)
                wx, g8xm, g8h, kfx = w
                ps = psum_pool.tile([128, BL], f32, tag="ps")
                if is_m(jt):
                    for j in range(kfx // 2):
                        for bc in range(NBC):
                            lo = bc * BC
                            nc.tensor.matmul(
                                ps[:, lo : lo + BC],
                                g8xm[:, 2 * j : 2 * j + 2, :],
                                xq8_kp[j][:, :, lo : lo + BC],
                                start=(j == 0), stop=False, perf_mode=DR,
                            )
                    for k in range(kfx, KT):
                        lhsT = wx[:, (k - kfx) * 128 : (k - kfx + 1) * 128]
                        for bc in range(NBC):
                            lo = bc * BC
                            nc.tensor.matmul(
                                ps[:, lo : lo + BC], lhsT, rhs_x(k, bc),
                                start=(kfx == 0 and k == 0), stop=False,
                            )
                else:
                    for j in range(KT // 2):
                        for bc in range(NBC):
                            lo = bc * BC
                            nc.tensor.matmul(
                                ps[:, lo : lo + BC],
                                wx[:, 2 * j : 2 * j + 2, :],
                                xq8_kp[j][:, :, lo : lo + BC],
                                start=(j == 0), stop=False, perf_mode=DR,
                            )
                for j in range(KT // 2):
                    for bc in range(NBC):
                        lo = bc * BC
                        nc.tensor.matmul(
                            ps[:, lo : lo + BC],
                            g8h[:, 2 * j : 2 * j + 2, :],
                            hq8_kp[j][:, :, lo : lo + BC],
                            start=False, stop=(j == KT // 2 - 1),
                            perf_mode=DR,
                        )
                g = gate_pool.tile([128, BL], bf16, tag="g")
                cw = BL // chunk_act
                for a in range(chunk_act):
                    sl = slice(a * cw, (a + 1) * cw)
                    nc.scalar.activation(
                        g[:, sl], ps[:, sl], func,
                        bias=bias_sb[:, jt : jt + 1], scale=scl_sb[:, 0:1],
                    )
                return g

            def gate_pair(jt_a, jt_b, func_a, func_b, w_a, w_b):
                """Two fp8-only gate tiles with interleaved matmuls: each
                arriving fp8 k-pair feeds 8 instructions instead of 4, so
                the warmup PE consumption rate matches the HBM-limited
                k-pair arrival rate (uses both PSUM buffers)."""
                wxa, g8ha = w_a
                wxb, g8hb = w_b
                psa = psum_pool.tile([128, BL], f32, tag="ps")
                psb = psum_pool.tile([128, BL], f32, tag="ps")
                for j in range(KT // 2):
                    for ps_, wx_ in ((psa, wxa), (psb, wxb)):
                        for bc in range(NBC):
                            lo = bc * BC
                            nc.tensor.matmul(
                                ps_[:, lo : lo + BC],
                                wx_[:, 2 * j : 2 * j + 2, :],
                                xq8_kp[j][:, :, lo : lo + BC],
                                start=(j == 0), stop=False, perf_mode=DR,
                            )
                for j in range(KT // 2):
                    for ps_, g8h_ in ((psa, g8ha), (psb, g8hb)):
                        for bc in range(NBC):
                            lo = bc * BC
                            nc.tensor.matmul(
                                ps_[:, lo : lo + BC],
                                g8h_[:, 2 * j : 2 * j + 2, :],
                                hq8_kp[j][:, :, lo : lo + BC],
                                start=False, stop=(j == KT // 2 - 1),
                                perf_mode=DR,
                            )
                ga = gate_pool.tile([128, BL], bf16, tag="g")
                nc.scalar.activation(ga[:], psa[:], func_a,
                                     bias=bias_sb[:, jt_a : jt_a + 1],
                                     scale=scl_sb[:, 0:1])
                gb = gate_pool.tile([128, BL], bf16, tag="g")
                nc.scalar.activation(gb[:], psb[:], func_b,
                                     bias=bias_sb[:, jt_b : jt_b + 1],
                                     scale=scl_sb[:, 0:1])
                return ga, gb

            def hwp_tile(t, w_i=None, chunk_act=1):
                """hw_proj tile t (x-only, full bf16, unscaled)."""
                jt = 5 * NT + t
                if w_i is None:
                    w_i = wpool.tile([128, E], bf16, tag="w")
                    nc.gpsimd.dma_start(w_i[:], whwp[t])
                ps = psum_pool.tile([128, BL], f32, tag="ps")
                for k in range(KT):
                    lhsT = w_i[:, k * 128 : (k + 1) * 128]
                    for bc in range(NBC):
                        lo = bc * BC
                        nc.tensor.matmul(
                            ps[:, lo : lo + BC], lhsT, rhs_x(k, bc),
                            start=(k == 0), stop=(k == KT - 1),
                        )
                g = gate_pool.tile([128, BL], bf16, tag="g")
                cw = BL // chunk_act
                for a in range(chunk_act):
                    sl = slice(a * cw, (a + 1) * cw)
                    nc.scalar.activation(
                        g[:, sl], ps[:, sl], AF.Identity,
                        bias=bias_sb[:, jt : jt + 1]
                    )
                return g

            mult, addop, subop = (
                mybir.AluOpType.mult,
                mybir.AluOpType.add,
                mybir.AluOpType.subtract,
            )

            w_hwp_l = None
            for t in range(NT):
                last = t == NT - 1
                if t == 0:
                    # group 0, fp8-first: o/hw/i/f need only the quantized
                    # activations, giving the x bf16 stream (m gate, hwp)
                    # the longest runway.
                    o_g = gate_tile(3 * NT, AF.Sigmoid, w=w_o0)
                    # xt k4-7 (1st needed by m0 ~40us) issue here, between
                    # activations: they stay out of the fp8 stream's way
                    # during the critical 8-28us warmup window.
                    for k in range(4, 6):
                        nc.scalar.dma_start(xt_k[k][:],
                                            xT[k * 128 : (k + 1) * 128, :])
                    hw_g = gate_tile(4 * NT, AF.Sigmoid, w=w_hw0)
                    for k in range(6, KT):
                        nc.scalar.dma_start(xt_k[k][:],
                                            xT[k * 128 : (k + 1) * 128, :])
                    i_g = gate_tile(0, AF.Sigmoid, w=w_i0)
                    f_g = gate_tile(NT, AF.Sigmoid, w=w_f0)
                    m_g = gate_tile(2 * NT, AF.Tanh, w=w_m0)
                else:
                    if not last:
                        hwp = hwp_tile(t)
                    i_g = gate_tile(t, AF.Sigmoid)
                    m_g = gate_tile(2 * NT + t, AF.Tanh)
                    f_g = gate_tile(NT + t, AF.Sigmoid)

                ct = cpool.tile([128, BL], bf16, tag="c")
                nc.gpsimd.dma_start(ct[:], cT[t * 128 : (t + 1) * 128, :])

                t1 = tmp_pool.tile([128, BL], bf16, tag="tmp")
                nc.vector.tensor_tensor(t1[:], i_g[:], m_g[:], mult)
                t2 = tmp_pool.tile([128, BL], bf16, tag="tmp")
                nc.vector.tensor_tensor(t2[:], f_g[:], ct[:], mult)
                mem = out_pool.tile([128, BL], bf16, tag="mem")
                nc.vector.tensor_tensor(mem[:], t1[:], t2[:], addop)
                nc.sync.dma_start(memT[t * 128 : (t + 1) * 128, :], mem[:])

                if t != 0:
                    o_g = gate_tile(3 * NT + t, AF.Sigmoid)

                tmem = tmp_pool.tile([128, BL], bf16, tag="tmp")
                nc.scalar.activation(tmem[:], mem[:], AF.Tanh)
                outp = tmp_pool.tile([128, BL], bf16, tag="tmp")
                nc.vector.tensor_tensor(outp[:], o_g[:], tmem[:], mult)

                if t != 0:
                    hw_g = gate_tile(4 * NT + t, AF.Sigmoid,
                                     chunk_act=4 if last else 1)
                if t == 0:
                    hwp = hwp_tile(0, w_i=w_hwp0)
                if t == NT - 2:
                    # prefetch the last group's hwp weights
                    w_hwp_l = wpool.tile([128, E], bf16, tag="w")
                    nc.gpsimd.dma_start(w_hwp_l[:], whwp[NT - 1])

                if not last:
                    # out = hwp + hw*(outp - hwp), chunked so the blend
                    # pipelines with the output DMA.
                    u = tmp_pool.tile([128, BL], bf16, tag="tmp")
                    nc.vector.tensor_tensor(u[:], outp[:], hwp[:], subop)
                    for e in range(2):
                        sl = slice(e * (BL // 2), (e + 1) * (BL // 2))
                        v = tmp_pool.tile([128, BL // 2], bf16, tag="v")
                        nc.vector.tensor_tensor(v[:], hw_g[:, sl], u[:, sl], mult)
                        outf = out_pool.tile([128, BL // 2], bf16, tag="out")
                        nc.vector.tensor_tensor(outf[:], v[:], hwp[:, sl], addop)
                        nc.sync.dma_start(outT[t * 128 : (t + 1) * 128, sl], outf[:])
                else:
                    # Last group: hwp computed LAST in per-bc chunks with
                    # fused blend+DMA; final 512 cols split into two 256-col
                    # chunks so the post-matmul tail is minimal.
                    jt = 5 * NT + t
                    chunks = [(0, BC), (BC, BC), (2 * BC, BC),
                              (3 * BC, BC // 2), (3 * BC + BC // 2, BC // 2)]
                    for ci, (lo, cw) in enumerate(chunks):
                        sl = slice(lo, lo + cw)
                        ps = psum_pool.tile([128, cw], f32, tag="ps")
                        for k in range(KT):
                            nc.tensor.matmul(
                                ps[:],
                                w_hwp_l[:, k * 128 : (k + 1) * 128],
                                rhs_x(k, lo // BC)[:, lo % BC : lo % BC + cw],
                                start=(k == 0),
                                stop=(k == KT - 1),
                            )
                        # alternate blend lanes vector/gpsimd so the final
                        # chunks' element-wise chains run in parallel. The
                        # hwp bias-add needs a PSUM port (gpsimd has none):
                        # vector serves its own lane, scalar (free by now)
                        # serves the gpsimd lane, so the lanes never
                        # serialize on one engine.
                        ve = nc.vector if ci % 2 == 0 else nc.gpsimd
                        hwp_c = tmp_pool.tile([128, cw], bf16, tag="v")
                        if ci % 2 == 0:
                            nc.vector.tensor_scalar_add(
                                hwp_c[:], ps[:], bias_sb[:, jt : jt + 1]
                            )
                        else:
                            nc.scalar.activation(
                                hwp_c[:], ps[:], AF.Identity,
                                bias=bias_sb[:, jt : jt + 1],
                            )
                        u = tmp_pool.tile([128, cw], bf16, tag="v")
                        ve.tensor_tensor(u[:], outp[:, sl], hwp_c[:], subop)
                        v = tmp_pool.tile([128, cw], bf16, tag="v")
                        ve.tensor_tensor(v[:], hw_g[:, sl], u[:], mult)
                        outf = out_pool.tile([128, cw], bf16, tag="out")
                        ve.tensor_tensor(outf[:], v[:], hwp_c[:], addop)
                        nc.sync.dma_start(outT[t * 128 : (t + 1) * 128, sl], outf[:])

    nc.compile()
    return nc


_NC_CACHE = None


def _get_nc():
    global _NC_CACHE
    if _NC_CACHE is None:
        _NC_CACHE = build_nc()
    return _NC_CACHE


def _pack_weights(W, njt):
    # W [njt*128 j, K e] -> [njt, 128 p, K] with [jt, p, k*128+m] = W[jt*128+m, k*128+p]
    W = np.asarray(W, np.float32)
    K = W.shape[1]
    kt = K // 128
    return np.ascontiguousarray(
        W.reshape(njt, 128, kt, 128).transpose(0, 3, 2, 1).reshape(njt, 128, K)
    )


def _q8(a, sc):
    return np.clip(np.asarray(a, np.float32) * sc, -240.0, 240.0).astype(F8E4)


def prepare_in_maps(x, h, c, Wi, bi, Ws, bs):
    x = np.asarray(x, np.float32)
    h = np.asarray(h, np.float32)
    Wi = np.asarray(Wi, np.float32)
    Ws = np.asarray(Ws, np.float32)
    Wg = Wi[: 5 * H]

    bias_comb = np.concatenate(
        [np.asarray(bi[: 5 * H], np.float32) + np.asarray(bs, np.float32),
         np.asarray(bi[5 * H :], np.float32)]
    )
    bias_pack = np.ascontiguousarray(bias_comb.reshape(NJI, 128).T).astype(np.float32)
    whwp_p = _pack_weights(Wi[5 * H :], NT).astype(BF16)

    sW = np.float32(224.0 / max(np.abs(Wg).max(), np.abs(Ws).max()))
    sA = np.float32(224.0 / max(np.abs(x).max(), np.abs(h).max()))
    S = np.float32(sW * sA)
    # [jt, m, j, p] -> [jt, p, j, m]
    wq8x_p = _q8(Wg.reshape(NJS, 128, KT, 128).transpose(0, 3, 2, 1), sW)
    wq8h_p = _q8(Ws.reshape(NJS, 128, KT, 128).transpose(0, 3, 2, 1), sW)
    wbxm_p = np.ascontiguousarray(
        _pack_weights(Wg[2 * H : 3 * H] * S, NT)).astype(BF16)
    common = {
        "bias": bias_pack,
        "whwp": whwp_p,
        "wq8x": np.ascontiguousarray(wq8x_p),
        "wq8h": np.ascontiguousarray(wq8h_p),
        "wbxm": wbxm_p,
        "scl": np.full((128, 1), 1.0 / S, np.float32),
    }

    in_maps = []
    for i in range(N_CORES):
        s = slice(i * BL, (i + 1) * BL)
        m = {
            "xT": np.ascontiguousarray(x[s].T).astype(BF16),
            "cT": np.ascontiguousarray(np.asarray(c[s], np.float32).T).astype(BF16),
            # [p, j, b] with value in[b, j*128+p]
            "xq8": np.ascontiguousarray(
                _q8(x[s].T.reshape(KT, 128, BL).transpose(1, 0, 2), sA)),
            "hq8": np.ascontiguousarray(
                _q8(h[s].T.reshape(KT, 128, BL).transpose(1, 0, 2), sA)),
        }
        m.update(common)
        in_maps.append(m)
    return in_maps


def run(in_maps, trace=False):
    nc = _get_nc()
    res = run_bass_kernel_spmd(nc, in_maps, core_ids=list(range(N_CORES)), trace=trace)
    out = np.empty((B, H), np.float32)
    mem = np.empty((B, H), np.float32)
    for i in range(N_CORES):
        s = slice(i * BL, (i + 1) * BL)
        out[s] = res.results[i]["outT"].astype(np.float32).T
        mem[s] = res.results[i]["memT"].astype(np.float32).T
    return (out, mem), res


def kernel(x, h, c, Wi, bi, Ws, bs):
    in_maps = prepare_in_maps(x, h, c, Wi, bi, Ws, bs)
    (out, mem), _ = run(in_maps, trace=False)
    return out, mem
